# revision 62
# baseline (speedup 1.0000x reference)
"""Causal self-attention on 8 Trainium2 NeuronCores (Bass/Tile).

Problem shape (hardcoded): x [2, 2048, 1024], W_attn [1024, 3072],
b_attn [3072], W_proj [1024, 1024], b_proj [1024], 16 heads, hd=64.

Sharding: tensor-parallel over (batch, head-group). Core k handles
batch k//4 and heads 4*(k%4) .. 4*(k%4)+3 (two head-pairs). Each core
computes its 4 heads' attention and a partial output projection
(y_local @ W_proj[rows]) of shape [2048, 1024]; the host sums the four
partials per batch and adds b_proj.

v7 design (157us -> 121us on the TimelineSim cost model):

- P@V runs in natural [q, d] layout: out tiles are [128 q-partitions,
  65 moving cols] instead of the transposed [65 partitions, 512 moving].
  PE cost is paid per moving column, so filling all 128 output
  partitions halves the P@V time (65 vs 128 cycles per 128 q x 128 k
  block). A 4-head accumulator tile [128, 4, 65] shares one PSUM bank
  via a single start/stop accumulation group spanning all heads.
- The 65th V column is ones, so softmax sums land in a PSUM *column*;
  normalization is per-partition work: DVE reciprocal [128,4,1], DVE
  tensor_scalar multiply, then a 128-cycle PE transpose (rhs identity)
  rebuilds y^T for the projection lhsT. No DMA round-trips.
- Emission order = scheduler priority: S^T/exp of strip s+1 is emitted
  above PV(s) and proj(s-1) so the PE keeps the (saturated) Act engine
  fed with S^T tiles; QKV chains alternate between two single-buffer
  PSUM tags as stall filler.
- PSUM tags never mix tile shapes with bufs>=2 (empirically corrupts
  under this stack); single-slot tags serialize safely.
- Emission order per slot: QKV(s+1), S(s+1), proj(s-1), PV(s) — feeders
  (QKV -> S -> exp) outrank sinks; proj above PV frees the shared
  qkv/yt PSUM slots for the next strip's QKV chains.
- Last strip: per-pair PV split, normalize-from-PSUM shortcut, DVE/Act
  eviction split, and out-DMAs on two DGE queues to shorten the tail.
"""

import sys

for _p in ("/opt/trn_rl_repo", "/root/.axon_site/_ro/trn_rl_repo"):
    if _p not in sys.path:
        sys.path.insert(0, _p)

import ml_dtypes
import numpy as np

import concourse.bass as bass  # noqa: F401  (engine types)
import concourse.mybir as mybir
import concourse.tile as tile
from concourse import bacc
from concourse.bass_utils import run_bass_kernel_spmd

F32 = mybir.dt.float32
BF16 = mybir.dt.bfloat16
NP_BF16 = ml_dtypes.bfloat16

B = 2
T = 2048
C = 1024
H = 16
HD = 64
NCORES = 8
HEADS_PER_CORE = 4  # two pairs
PAIRS = 2
NKT = T // 128       # 16 k-tiles per head
NST = T // 512       # 4 q-strips per head
CKT = C // 128       # 8 contraction tiles for C

_CACHE = {}


def _build():
    """Build the SPMD Bass program (identical for all cores)."""
    nc = bacc.Bacc(None, target_bir_lowering=False)

    # x^T pre-tiled on host: [p, kc, t] = x^T[kc*128+p, t]
    xt_d = nc.dram_tensor("xt", [128, CKT, T], BF16, kind="ExternalInput")
    wq_d = nc.dram_tensor("wq", [128, PAIRS, CKT, 128], BF16, kind="ExternalInput")
    wk_d = nc.dram_tensor("wk", [128, PAIRS, CKT, 128], BF16, kind="ExternalInput")
    wv_d = nc.dram_tensor("wv", [128, CKT, 256], BF16, kind="ExternalInput")
    wp_d = nc.dram_tensor("wp", [128, 2, C], BF16, kind="ExternalInput")
    mask_d = nc.dram_tensor("mask", [128, 128], BF16, kind="ExternalInput")
    ones_d = nc.dram_tensor("ones", [128, 1], BF16, kind="ExternalInput")
    ident_d = nc.dram_tensor("ident", [128, 128], BF16, kind="ExternalInput")
    out_d = nc.dram_tensor("out", [T, C], BF16, kind="ExternalOutput")

    with tile.TileContext(nc) as tc, (
        tc.tile_pool(name="const", bufs=1)
    ) as const, (
        tc.tile_pool(name="weights", bufs=1)
    ) as wpool, (
        tc.tile_pool(name="acts", bufs=1)
    ) as apool, (
        tc.tile_pool(name="xstream", bufs=3)
    ) as xpool, (
        tc.tile_pool(name="ptp", bufs=3)
    ) as ppool, (
        tc.tile_pool(name="evict", bufs=3)
    ) as epool, (
        tc.tile_pool(name="st_ps", bufs=1, space="PSUM")
    ) as st_ps, (
        tc.tile_pool(name="y_ps", bufs=1, space="PSUM")
    ) as y_ps, (
        tc.tile_pool(name="qkv_ps", bufs=1, space="PSUM")
    ) as qkv_ps:
        mask_tri = const.tile([128, 128], BF16)
        ident = const.tile([128, 128], BF16)

        wq = wpool.tile([128, PAIRS, CKT, 128], BF16)
        wk = wpool.tile([128, PAIRS, CKT, 128], BF16)
        wv = wpool.tile([128, CKT, 256], BF16)
        wp = wpool.tile([128, 2, C], BF16)

        # activations kept resident in SBUF
        qt = apool.tile([128, PAIRS, T], BF16)   # q^T, heads stacked in pairs
        kt = apool.tile([128, PAIRS, T], BF16)   # k^T
        v_nat = apool.tile([128, NKT, HEADS_PER_CORE, HD + 1], BF16)
        ytn = apool.tile([128, PAIRS, T], BF16)  # normalized y^T

        # ---- lead-in DMAs: first strip of x + pair-0 weights first ----
        xs0 = xpool.tile([128, CKT, 512], BF16, name="xc_0", tag="xc")
        nc.scalar.dma_start(wq[:, 0, 0:1], wq_d[:, 0, 0:1])
        nc.sync.dma_start(xs0[:, 0:1], xt_d[:, 0:1, 0:512])
        nc.sync.dma_start(xs0[:, 1:2], xt_d[:, 1:2, 0:512])
        nc.scalar.dma_start(wq[:, 0, 1:4], wq_d[:, 0, 1:4])
        nc.sync.dma_start(xs0[:, 2:4], xt_d[:, 2:4, 0:512])
        nc.scalar.dma_start(wq[:, 0, 4:8], wq_d[:, 0, 4:8])
        nc.sync.dma_start(xs0[:, 4:8], xt_d[:, 4:8, 0:512])
        nc.sync.dma_start(wv[:], wv_d[:])
        nc.gpsimd.dma_start(wk[:, 0], wk_d[:, 0])
        nc.scalar.dma_start(mask_tri[:], mask_d[:])
        nc.scalar.dma_start(ident[:], ident_d[:])
        nc.gpsimd.dma_start(wq[:, 1], wq_d[:, 1])
        nc.gpsimd.dma_start(wk[:, 1], wk_d[:, 1])
        nc.scalar.dma_start(wp[:], wp_d[:])
        # ones column of v_nat (the 65th rhs column yields softmax sums)
        for hh in range(HEADS_PER_CORE):
            nc.sync.dma_start(
                v_nat[:, :, hh, HD:HD + 1], ones_d[:].to_broadcast((128, NKT, 1))
            )

        # warm the Exp table on Act while lead-in DMAs are in flight
        warm = ppool.tile([1, 2], F32, name="warm", tag="warm", bufs=1)
        nc.scalar.activation(
            warm[:], mask_tri[0:1, 0:2], mybir.ActivationFunctionType.Exp
        )

        # ---- QKV (q^T/k^T transposed; v natural) ----
        def fetch_x(s):
            xs = xpool.tile([128, CKT, 512], BF16, name=f"xc_{s}", tag="xc")
            nc.sync.dma_start(xs[:], xt_d[:, :, s * 512:(s + 1) * 512])
            return xs

        def emit_qkv(s, xs):
            evict = nc.scalar.copy if s == 0 else nc.vector.tensor_copy
            slots = [(qkv_ps, "qkv", 1), (st_ps, "yt", 1)]
            snext = iter(range(100))
            if xs is None:
                xs = fetch_x(s)
            def qk_chain(p, w_t, dest):
                pool_, tag_, bufs_ = slots[next(snext) % len(slots)]
                ps = pool_.tile(
                    [128, 512], F32,
                    name=f"qkps_{s}_{p}_{0 if w_t is wq else 1}", tag=tag_,
                    bufs=bufs_,
                )
                for kc in range(CKT):
                    nc.tensor.matmul(
                        ps[:],
                        w_t[:, p, kc, :],
                        xs[:, kc, :],
                        start=(kc == 0),
                        stop=(kc == CKT - 1),
                    )
                evict(dest[:, p, s * 512:(s + 1) * 512], ps[:])

            def v_chain(i):
                t = 4 * s + i
                pool_, tag_, bufs_ = slots[next(snext) % len(slots)]
                psv = pool_.tile(
                    [128, 256], F32, name=f"vps_{s}_{i}", tag=tag_, bufs=bufs_
                )
                for kc in range(CKT):
                    nc.tensor.matmul(
                        psv[:],
                        xs[:, kc, i * 128:(i + 1) * 128],
                        wv[:, kc, :],
                        start=(kc == 0),
                        stop=(kc == CKT - 1),
                    )
                nc.vector.tensor_copy(
                    v_nat[:, t, :, 0:HD],
                    psv[:].rearrange("p (h d) -> p h d", h=HEADS_PER_CORE),
                )

            if s == 0:
                # pair 0 + its first v tiles first: strip-0 attention can
                # start while pair 1 is still projecting
                qk_chain(0, wq, qt)
                qk_chain(0, wk, kt)
                v_chain(0)
                v_chain(1)
                qk_chain(1, wq, qt)
                qk_chain(1, wk, kt)
                v_chain(2)
                v_chain(3)
            else:
                for p in range(PAIRS):
                    qk_chain(p, wq, qt)
                    qk_chain(p, wk, kt)
                for i in range(4):
                    v_chain(i)

        # ---- attention ----
        def emit_S(s):
            n_k = 4 * s + 4  # k-tiles for this strip (causal)
            ngrp = n_k // 2
            ptiles = {}
            # S-phase: S^T matmuls + exp, pairs alternating per group
            for g in range(ngrp):
                for p in range(PAIRS):
                    st = [
                        st_ps.tile(
                            [128, 1024], F32,
                            name=f"st_{s}_{g}_{p}_{h}", tag="st", bufs=2,
                        )
                        for h in range(2)
                    ]
                    # S^T matmuls, interleaved across heads for overlap
                    for jj in range(2):
                        j = 2 * g + jj
                        c0 = max(0, 128 * (j - 4 * s))
                        for h in range(2):
                            nc.tensor.matmul(
                                st[h][:, jj * 512 + c0:(jj + 1) * 512],
                                kt[h * HD:(h + 1) * HD, p, j * 128:(j + 1) * 128],
                                qt[h * HD:(h + 1) * HD, p, s * 512 + c0:(s + 1) * 512],
                                start=True,
                                stop=True,
                            )
                    # exp, cropped to the first valid column of the group
                    ec0 = max(0, 128 * (2 * g - 4 * s))
                    c0b = max(0, 128 * (2 * g + 1 - 4 * s))
                    for h in range(2):
                        ptile = ppool.tile(
                            [128, 1024], BF16,
                            name=f"pt_{s}_{p}_{g}_{h}", tag="pt", bufs=56,
                        )
                        ptiles[(p, g, h)] = ptile
                        if c0b >= 256:
                            # both blocks diagonal: separate cropped exps skip
                            # the dead middle columns
                            nc.scalar.activation(
                                ptile[:, ec0:512], st[h][:, ec0:512],
                                mybir.ActivationFunctionType.Exp,
                            )
                            nc.scalar.activation(
                                ptile[:, 512 + c0b:1024], st[h][:, 512 + c0b:1024],
                                mybir.ActivationFunctionType.Exp,
                            )
                        else:
                            nc.scalar.activation(
                                ptile[:, ec0:1024], st[h][:, ec0:1024],
                                mybir.ActivationFunctionType.Exp,
                            )
                        for jj in range(2):
                            j = 2 * g + jj
                            c0 = max(0, 128 * (j - 4 * s))
                            if j >= 4 * s:  # diagonal block: triangular mask
                                blk = ptile[:, jj * 512 + c0:jj * 512 + c0 + 128]
                                nc.gpsimd.tensor_mul(blk, blk, mask_tri[:])

            return ptiles

        def emit_PV(s, ptiles):
            return ptiles

        def emit_PV(s, ptiles):
            n_k = 4 * s + 4
            # PV-phase: natural-layout accumulation per q-subtile, then
            # per-partition normalize + PE transpose back to y^T.
            # The final subtile of the final strip runs per-pair so the
            # last ladder+proj chain is as short as possible.
            for ti in range(4):
                n_j = 4 * s + ti + 1
                pair_groups = (
                    [(0, 1)] if not (s == NST - 1 and ti == 3) else [(0,), (1,)]
                )
                for pg in pair_groups:
                    hhs = [2 * p + h for p in pg for h in range(2)]
                    y = y_ps.tile(
                        [128, len(hhs), HD + 1], F32,
                        name=f"y_{s}_{ti}_{pg[0]}", tag="y", bufs=2,
                    )
                    for yi, hh in enumerate(hhs):
                        p, h = hh // 2, hh % 2
                        for j in range(n_j):
                            g, jj = j // 2, j % 2
                            nc.tensor.matmul(
                                y[:, yi, :],
                                ptiles[(p, g, h)][
                                    :, jj * 512 + ti * 128:jj * 512 + (ti + 1) * 128
                                ],
                                v_nat[:, j, hh, :],
                                start=(yi == 0 and j == 0),
                                stop=(yi == len(hhs) - 1 and j == n_j - 1),
                                skip_group_check=True,
                            )
                    recip = ppool.tile(
                        [128, len(hhs), 1], F32,
                        name=f"rc_{s}_{ti}_{pg[0]}", tag="recip", bufs=8,
                    )
                    nc.vector.reciprocal(recip[:], y[:, :, HD:HD + 1])
                    ynorm = ppool.tile(
                        [128, len(hhs), HD], BF16,
                        name=f"yn_{s}_{ti}_{pg[0]}", tag="ynorm", bufs=8,
                    )
                    if len(pair_groups) > 1:
                        # tail subtile: normalize straight from PSUM
                        # (shorter chain; bank hold is moot at kernel end)
                        for yi in range(len(hhs)):
                            nc.vector.tensor_scalar_mul(
                                ynorm[:, yi, :], y[:, yi, 0:HD], recip[:, yi, :]
                            )
                    else:
                        yraw = ppool.tile(
                            [128, len(hhs), HD + 1], BF16,
                            name=f"yr_{s}_{ti}_{pg[0]}", tag="yraw", bufs=8,
                        )
                        nc.vector.tensor_copy(yraw[:], y[:])
                        for yi in range(len(hhs)):
                            nc.vector.tensor_scalar_mul(
                                ynorm[:, yi, :], yraw[:, yi, 0:HD], recip[:, yi, :]
                            )
                    for p in pg:
                        o = (2 * p - 2 * pg[0])
                        yt = st_ps.tile(
                            [128, 128], BF16,
                            name=f"yt_{s}_{p}_{ti}", tag="yt", bufs=1,
                        )
                        nc.tensor.transpose(
                            yt[:], ynorm[:, o:o + 2, :], ident[:]
                        )
                        nc.vector.tensor_copy(
                            ytn[:, p, s * 512 + ti * 128:s * 512 + (ti + 1) * 128],
                            yt[:],
                        )

        # ---- output projection (partial), interleaved per strip ----
        def emit_proj(s, slots=None, use_act=False):
            if slots is None:
                slots = [(qkv_ps, "qkv", 1), (st_ps, "yt", 1)]
            for ti in range(4):
                t = 4 * s + ti
                ot = epool.tile([128, 1024], BF16, name=f"ot_{t}", tag="ot", bufs=7)
                for n in range(2):
                    pool_, tag_, bufs_ = slots[(2 * ti + n) % len(slots)]
                    op = pool_.tile(
                        [128, 512], F32, name=f"op_{t}_{n}", tag=tag_, bufs=bufs_
                    )
                    for f in range(2):
                        nc.tensor.matmul(
                            op[:],
                            ytn[:, f, t * 128:(t + 1) * 128],
                            wp[:, f, n * 512:(n + 1) * 512],
                            start=(f == 0),
                            stop=(f == 1),
                        )
                    if use_act and n == 1:
                        nc.scalar.copy(ot[:, n * 512:(n + 1) * 512], op[:])
                    else:
                        nc.vector.tensor_copy(ot[:, n * 512:(n + 1) * 512], op[:])
                    if not use_act:
                        nc.sync.dma_start(
                            out_d[t * 128:(t + 1) * 128, n * 512:(n + 1) * 512],
                            ot[:, n * 512:(n + 1) * 512],
                        )
                if use_act:
                    dma = nc.sync.dma_start if ti % 2 == 0 else nc.scalar.dma_start
                    dma(out_d[t * 128:(t + 1) * 128, :], ot[:])

        # ---- skewed software pipeline ----
        # per slot: attention first (highest priority), next strip's QKV and
        # the previous strip's projection as PE stall-filler
        emit_qkv(0, xs0)
        xss = {1: fetch_x(1)} if NST > 1 else {}
        pts = {0: emit_S(0)}
        for s in range(NST):
            if s + 2 < NST:
                xss[s + 2] = fetch_x(s + 2)
            if s + 1 < NST:
                emit_qkv(s + 1, xss.pop(s + 1))
                pts[s + 1] = emit_S(s + 1)
            if s >= 1:
                emit_proj(s - 1)
            emit_PV(s, pts.pop(s))
        # final projection: rotate over the now-free attention PSUM slots so
        # the tail pipeline isn't serialized on a single bank
        emit_proj(
            NST - 1,
            # explicit per-chain slots: early t-tiles may use the y banks
            # (still draining), late t-tiles use only st/qkv banks
            slots=[
                (y_ps, "y", 2), (y_ps, "y", 2),
                (st_ps, "st", 2), (st_ps, "st", 2),
                (qkv_ps, "qkv", 1), (st_ps, "st", 2),
                (st_ps, "st", 2), (qkv_ps, "qkv", 1),
            ],
            use_act=True,
        )

    nc.compile()
    return nc


def _prep_inputs(x, W_attn, b_attn, W_proj):
    """Per-core input maps. Core k: batch k//4, head-group k%4."""
    assert np.allclose(b_attn, 0.0), "nonzero b_attn not supported by this kernel"
    scale = 1.0 / np.sqrt(np.float32(HD))

    mask = (np.arange(128)[:, None] <= np.arange(128)[None, :]).astype(NP_BF16)
    ones = np.ones((128, 1), dtype=NP_BF16)
    ident = np.eye(128, dtype=NP_BF16)

    def lhsT_tiles(w):
        # [C, 128] -> [128, CKT, 128] with [p, t, c] = w[t*128+p, c]
        return np.ascontiguousarray(w.reshape(CKT, 128, 128).transpose(1, 0, 2))

    in_maps = []
    for core in range(NCORES):
        b = core // 4
        g = core % 4
        heads = [4 * g + i for i in range(HEADS_PER_CORE)]
        # [128, CKT, T] with [p, kc, t] = x^T[kc*128+p, t]
        xt = np.ascontiguousarray(
            x[b].T.reshape(CKT, 128, T).transpose(1, 0, 2)
        ).astype(NP_BF16)

        def w_slice(base, hs, sc=1.0):
            cols = np.concatenate(
                [np.arange(base + h * HD, base + (h + 1) * HD) for h in hs]
            )
            return np.ascontiguousarray(W_attn[:, cols]) * sc

        wq = np.stack(
            [lhsT_tiles(w_slice(0, heads[2 * p:2 * p + 2], scale)) for p in range(PAIRS)], axis=1
        ).astype(NP_BF16)  # [128, PAIRS, CKT, 128]
        wk = np.stack(
            [lhsT_tiles(w_slice(C, heads[2 * p:2 * p + 2])) for p in range(PAIRS)], axis=1
        ).astype(NP_BF16)
        # v weights in natural rhs layout: [128, CKT, 256], [p, kc, d] = Wv[kc*128+p, d]
        wv_cols = w_slice(2 * C, heads)  # [C, 256]
        wv = np.ascontiguousarray(
            wv_cols.reshape(CKT, 128, 256).transpose(1, 0, 2)
        ).astype(NP_BF16)
        # W_proj rows for this head group: [256, C] -> [128, 2, C]
        wp_rows = W_proj[heads[0] * HD:(heads[-1] + 1) * HD, :]
        wp = np.ascontiguousarray(
            wp_rows.reshape(2, 128, C).transpose(1, 0, 2)
        ).astype(NP_BF16)

        in_maps.append(
            {
                "xt": np.ascontiguousarray(xt),
                "wq": np.ascontiguousarray(wq),
                "wk": np.ascontiguousarray(wk),
                "wv": np.ascontiguousarray(wv),
                "wp": np.ascontiguousarray(wp),
                "mask": np.ascontiguousarray(mask),
                "ones": ones,
                "ident": ident,
            }
        )
    return in_maps


def kernel(x, W_attn, b_attn, W_proj, b_proj, _want_results=False, _spmd_kwargs=None):
    x = np.asarray(x, dtype=np.float32)
    W_attn = np.asarray(W_attn, dtype=np.float32)
    b_attn = np.asarray(b_attn, dtype=np.float32)
    W_proj = np.asarray(W_proj, dtype=np.float32)
    b_proj = np.asarray(b_proj, dtype=np.float32)

    if "nc" not in _CACHE:
        _CACHE["nc"] = _build()
    nc = _CACHE["nc"]

    in_maps = _prep_inputs(x, W_attn, b_attn, W_proj)
    kw = dict(_spmd_kwargs or {})
    res = run_bass_kernel_spmd(nc, in_maps, list(range(NCORES)), **kw)

    out = np.zeros((B, T, C), dtype=np.float32)
    for core in range(NCORES):
        out[core // 4] += np.asarray(res.results[core]["out"], dtype=np.float32)
    out += b_proj[None, None, :]
    if _want_results:
        return out, res
    return out


# revision 63
# speedup vs baseline: 1.0034x; 1.0034x over previous
"""Causal self-attention on 8 Trainium2 NeuronCores (Bass/Tile).

Problem shape (hardcoded): x [2, 2048, 1024], W_attn [1024, 3072],
b_attn [3072], W_proj [1024, 1024], b_proj [1024], 16 heads, hd=64.

Sharding: tensor-parallel over (batch, head-group). Core k handles
batch k//4 and heads 4*(k%4) .. 4*(k%4)+3 (two head-pairs). Each core
computes its 4 heads' attention and a partial output projection
(y_local @ W_proj[rows]) of shape [2048, 1024]; the host sums the four
partials per batch and adds b_proj.

v7 design (157us -> 121us on the TimelineSim cost model):

- P@V runs in natural [q, d] layout: out tiles are [128 q-partitions,
  65 moving cols] instead of the transposed [65 partitions, 512 moving].
  PE cost is paid per moving column, so filling all 128 output
  partitions halves the P@V time (65 vs 128 cycles per 128 q x 128 k
  block). A 4-head accumulator tile [128, 4, 65] shares one PSUM bank
  via a single start/stop accumulation group spanning all heads.
- The 65th V column is ones, so softmax sums land in a PSUM *column*;
  normalization is per-partition work: DVE reciprocal [128,4,1], DVE
  tensor_scalar multiply, then a 128-cycle PE transpose (rhs identity)
  rebuilds y^T for the projection lhsT. No DMA round-trips.
- Emission order = scheduler priority: S^T/exp of strip s+1 is emitted
  above PV(s) and proj(s-1) so the PE keeps the (saturated) Act engine
  fed with S^T tiles; QKV chains alternate between two single-buffer
  PSUM tags as stall filler.
- PSUM tags never mix tile shapes with bufs>=2 (empirically corrupts
  under this stack); single-slot tags serialize safely.
- Emission order per slot: QKV(s+1), S(s+1), proj(s-1), PV(s) — feeders
  (QKV -> S -> exp) outrank sinks; proj above PV frees the shared
  qkv/yt PSUM slots for the next strip's QKV chains.
- Last strip: per-pair PV split, normalize-from-PSUM shortcut, DVE/Act
  eviction split, and out-DMAs on two DGE queues to shorten the tail.
"""

import sys

for _p in ("/opt/trn_rl_repo", "/root/.axon_site/_ro/trn_rl_repo"):
    if _p not in sys.path:
        sys.path.insert(0, _p)

import ml_dtypes
import numpy as np

import concourse.bass as bass  # noqa: F401  (engine types)
import concourse.mybir as mybir
import concourse.tile as tile
from concourse import bacc
from concourse.bass_utils import run_bass_kernel_spmd

F32 = mybir.dt.float32
BF16 = mybir.dt.bfloat16
NP_BF16 = ml_dtypes.bfloat16

B = 2
T = 2048
C = 1024
H = 16
HD = 64
NCORES = 8
HEADS_PER_CORE = 4  # two pairs
PAIRS = 2
NKT = T // 128       # 16 k-tiles per head
NST = T // 512       # 4 q-strips per head
CKT = C // 128       # 8 contraction tiles for C

_CACHE = {}


def _build():
    """Build the SPMD Bass program (identical for all cores)."""
    nc = bacc.Bacc(None, target_bir_lowering=False)

    # x^T pre-tiled on host: [p, kc, t] = x^T[kc*128+p, t]
    xt_d = nc.dram_tensor("xt", [128, CKT, T], BF16, kind="ExternalInput")
    wq_d = nc.dram_tensor("wq", [128, PAIRS, CKT, 128], BF16, kind="ExternalInput")
    wk_d = nc.dram_tensor("wk", [128, PAIRS, CKT, 128], BF16, kind="ExternalInput")
    wv_d = nc.dram_tensor("wv", [128, CKT, 256], BF16, kind="ExternalInput")
    wp_d = nc.dram_tensor("wp", [128, 2, C], BF16, kind="ExternalInput")
    mask_d = nc.dram_tensor("mask", [128, 128], BF16, kind="ExternalInput")
    ones_d = nc.dram_tensor("ones", [128, 1], BF16, kind="ExternalInput")
    ident_d = nc.dram_tensor("ident", [128, 128], BF16, kind="ExternalInput")
    out_d = nc.dram_tensor("out", [T, C], BF16, kind="ExternalOutput")

    with tile.TileContext(nc) as tc, (
        tc.tile_pool(name="const", bufs=1)
    ) as const, (
        tc.tile_pool(name="weights", bufs=1)
    ) as wpool, (
        tc.tile_pool(name="acts", bufs=1)
    ) as apool, (
        tc.tile_pool(name="xstream", bufs=3)
    ) as xpool, (
        tc.tile_pool(name="ptp", bufs=3)
    ) as ppool, (
        tc.tile_pool(name="evict", bufs=3)
    ) as epool, (
        tc.tile_pool(name="st_ps", bufs=1, space="PSUM")
    ) as st_ps, (
        tc.tile_pool(name="y_ps", bufs=1, space="PSUM")
    ) as y_ps, (
        tc.tile_pool(name="qkv_ps", bufs=1, space="PSUM")
    ) as qkv_ps:
        mask_tri = const.tile([128, 128], BF16)
        ident = const.tile([128, 128], BF16)

        wq = wpool.tile([128, PAIRS, CKT, 128], BF16)
        wk = wpool.tile([128, PAIRS, CKT, 128], BF16)
        wv = wpool.tile([128, CKT, 256], BF16)
        wp = wpool.tile([128, 2, C], BF16)

        # activations kept resident in SBUF
        qt = apool.tile([128, PAIRS, T], BF16)   # q^T, heads stacked in pairs
        kt = apool.tile([128, PAIRS, T], BF16)   # k^T
        v_nat = apool.tile([128, NKT, HEADS_PER_CORE, HD + 1], BF16)
        ytn = apool.tile([128, PAIRS, T], BF16)  # normalized y^T

        # ---- lead-in DMAs: first strip of x + pair-0 weights first ----
        xs0 = xpool.tile([128, CKT, 512], BF16, name="xc_0", tag="xc")
        nc.scalar.dma_start(wq[:, 0, 0:1], wq_d[:, 0, 0:1])
        nc.sync.dma_start(xs0[:, 0:1], xt_d[:, 0:1, 0:512])
        nc.sync.dma_start(xs0[:, 1:2], xt_d[:, 1:2, 0:512])
        nc.scalar.dma_start(wq[:, 0, 1:4], wq_d[:, 0, 1:4])
        nc.sync.dma_start(xs0[:, 2:4], xt_d[:, 2:4, 0:512])
        nc.scalar.dma_start(wq[:, 0, 4:8], wq_d[:, 0, 4:8])
        nc.sync.dma_start(xs0[:, 4:8], xt_d[:, 4:8, 0:512])
        nc.sync.dma_start(wv[:], wv_d[:])
        nc.gpsimd.dma_start(wk[:, 0], wk_d[:, 0])
        nc.scalar.dma_start(mask_tri[:], mask_d[:])
        nc.scalar.dma_start(ident[:], ident_d[:])
        nc.gpsimd.dma_start(wq[:, 1], wq_d[:, 1])
        nc.gpsimd.dma_start(wk[:, 1], wk_d[:, 1])
        nc.scalar.dma_start(wp[:], wp_d[:])
        # ones column of v_nat (the 65th rhs column yields softmax sums)
        for hh in range(HEADS_PER_CORE):
            nc.sync.dma_start(
                v_nat[:, :, hh, HD:HD + 1], ones_d[:].to_broadcast((128, NKT, 1))
            )

        # warm the Exp table on Act while lead-in DMAs are in flight
        warm = ppool.tile([1, 2], F32, name="warm", tag="warm", bufs=1)
        nc.scalar.activation(
            warm[:], mask_tri[0:1, 0:2], mybir.ActivationFunctionType.Exp
        )

        # ---- QKV (q^T/k^T transposed; v natural) ----
        def fetch_x(s):
            xs = xpool.tile([128, CKT, 512], BF16, name=f"xc_{s}", tag="xc")
            nc.sync.dma_start(xs[:], xt_d[:, :, s * 512:(s + 1) * 512])
            return xs

        def emit_qkv(s, xs):
            evict = nc.scalar.copy if s == 0 else nc.vector.tensor_copy
            slots = [(qkv_ps, "qkv", 1), (st_ps, "yt", 1)]
            snext = iter(range(100))
            if xs is None:
                xs = fetch_x(s)
            def qk_chain(p, w_t, dest):
                pool_, tag_, bufs_ = slots[next(snext) % len(slots)]
                ps = pool_.tile(
                    [128, 512], F32,
                    name=f"qkps_{s}_{p}_{0 if w_t is wq else 1}", tag=tag_,
                    bufs=bufs_,
                )
                for kc in range(CKT):
                    nc.tensor.matmul(
                        ps[:],
                        w_t[:, p, kc, :],
                        xs[:, kc, :],
                        start=(kc == 0),
                        stop=(kc == CKT - 1),
                    )
                evict(dest[:, p, s * 512:(s + 1) * 512], ps[:])

            def v_chain(i):
                t = 4 * s + i
                pool_, tag_, bufs_ = slots[next(snext) % len(slots)]
                psv = pool_.tile(
                    [128, 256], F32, name=f"vps_{s}_{i}", tag=tag_, bufs=bufs_
                )
                for kc in range(CKT):
                    nc.tensor.matmul(
                        psv[:],
                        xs[:, kc, i * 128:(i + 1) * 128],
                        wv[:, kc, :],
                        start=(kc == 0),
                        stop=(kc == CKT - 1),
                    )
                nc.vector.tensor_copy(
                    v_nat[:, t, :, 0:HD],
                    psv[:].rearrange("p (h d) -> p h d", h=HEADS_PER_CORE),
                )

            if s == 0:
                # pair 0 + its first v tiles first: strip-0 attention can
                # start while pair 1 is still projecting
                qk_chain(0, wq, qt)
                qk_chain(0, wk, kt)
                v_chain(0)
                v_chain(1)
                qk_chain(1, wq, qt)
                qk_chain(1, wk, kt)
                v_chain(2)
                v_chain(3)
            else:
                for p in range(PAIRS):
                    qk_chain(p, wq, qt)
                    qk_chain(p, wk, kt)
                for i in range(4):
                    v_chain(i)

        # ---- attention ----
        def emit_S(s):
            n_k = 4 * s + 4  # k-tiles for this strip (causal)
            ngrp = n_k // 2
            ptiles = {}
            # S-phase: S^T matmuls + exp, pairs alternating per group
            for g in range(ngrp):
                for p in range(PAIRS):
                    st = [
                        st_ps.tile(
                            [128, 1024], F32,
                            name=f"st_{s}_{g}_{p}_{h}", tag="st", bufs=2,
                        )
                        for h in range(2)
                    ]
                    # S^T matmuls, interleaved across heads for overlap
                    for jj in range(2):
                        j = 2 * g + jj
                        c0 = max(0, 128 * (j - 4 * s))
                        for h in range(2):
                            nc.tensor.matmul(
                                st[h][:, jj * 512 + c0:(jj + 1) * 512],
                                kt[h * HD:(h + 1) * HD, p, j * 128:(j + 1) * 128],
                                qt[h * HD:(h + 1) * HD, p, s * 512 + c0:(s + 1) * 512],
                                start=True,
                                stop=True,
                            )
                    # exp, cropped to the first valid column of the group
                    ec0 = max(0, 128 * (2 * g - 4 * s))
                    c0b = max(0, 128 * (2 * g + 1 - 4 * s))
                    for h in range(2):
                        ptile = ppool.tile(
                            [128, 1024], BF16,
                            name=f"pt_{s}_{p}_{g}_{h}", tag="pt", bufs=56,
                        )
                        ptiles[(p, g, h)] = ptile
                        if c0b >= 256:
                            # both blocks diagonal: separate cropped exps skip
                            # the dead middle columns
                            nc.scalar.activation(
                                ptile[:, ec0:512], st[h][:, ec0:512],
                                mybir.ActivationFunctionType.Exp,
                            )
                            nc.scalar.activation(
                                ptile[:, 512 + c0b:1024], st[h][:, 512 + c0b:1024],
                                mybir.ActivationFunctionType.Exp,
                            )
                        else:
                            nc.scalar.activation(
                                ptile[:, ec0:1024], st[h][:, ec0:1024],
                                mybir.ActivationFunctionType.Exp,
                            )
                        for jj in range(2):
                            j = 2 * g + jj
                            c0 = max(0, 128 * (j - 4 * s))
                            if j >= 4 * s:  # diagonal block: triangular mask
                                blk = ptile[:, jj * 512 + c0:jj * 512 + c0 + 128]
                                mul = (
                                    nc.vector.tensor_mul if s == NST - 1
                                    else nc.gpsimd.tensor_mul
                                )
                                mul(blk, blk, mask_tri[:])

            return ptiles

        def emit_PV(s, ptiles):
            return ptiles

        def emit_PV(s, ptiles):
            n_k = 4 * s + 4
            # PV-phase: natural-layout accumulation per q-subtile, then
            # per-partition normalize + PE transpose back to y^T.
            # The final subtile of the final strip runs per-pair so the
            # last ladder+proj chain is as short as possible.
            for ti in range(4):
                n_j = 4 * s + ti + 1
                pair_groups = (
                    [(0, 1)] if not (s == NST - 1 and ti == 3) else [(0,), (1,)]
                )
                for pg in pair_groups:
                    hhs = [2 * p + h for p in pg for h in range(2)]
                    y = y_ps.tile(
                        [128, len(hhs), HD + 1], F32,
                        name=f"y_{s}_{ti}_{pg[0]}", tag="y", bufs=2,
                    )
                    for yi, hh in enumerate(hhs):
                        p, h = hh // 2, hh % 2
                        for j in range(n_j):
                            g, jj = j // 2, j % 2
                            nc.tensor.matmul(
                                y[:, yi, :],
                                ptiles[(p, g, h)][
                                    :, jj * 512 + ti * 128:jj * 512 + (ti + 1) * 128
                                ],
                                v_nat[:, j, hh, :],
                                start=(yi == 0 and j == 0),
                                stop=(yi == len(hhs) - 1 and j == n_j - 1),
                                skip_group_check=True,
                            )
                    recip = ppool.tile(
                        [128, len(hhs), 1], F32,
                        name=f"rc_{s}_{ti}_{pg[0]}", tag="recip", bufs=8,
                    )
                    nc.vector.reciprocal(recip[:], y[:, :, HD:HD + 1])
                    ynorm = ppool.tile(
                        [128, len(hhs), HD], BF16,
                        name=f"yn_{s}_{ti}_{pg[0]}", tag="ynorm", bufs=8,
                    )
                    if len(pair_groups) > 1:
                        # tail subtile: normalize straight from PSUM
                        # (shorter chain; bank hold is moot at kernel end)
                        for yi in range(len(hhs)):
                            nc.vector.tensor_scalar_mul(
                                ynorm[:, yi, :], y[:, yi, 0:HD], recip[:, yi, :]
                            )
                    else:
                        yraw = ppool.tile(
                            [128, len(hhs), HD + 1], BF16,
                            name=f"yr_{s}_{ti}_{pg[0]}", tag="yraw", bufs=8,
                        )
                        nc.vector.tensor_copy(yraw[:], y[:])
                        for yi in range(len(hhs)):
                            nc.vector.tensor_scalar_mul(
                                ynorm[:, yi, :], yraw[:, yi, 0:HD], recip[:, yi, :]
                            )
                    for p in pg:
                        o = (2 * p - 2 * pg[0])
                        yt = st_ps.tile(
                            [128, 128], BF16,
                            name=f"yt_{s}_{p}_{ti}", tag="yt", bufs=1,
                        )
                        nc.tensor.transpose(
                            yt[:], ynorm[:, o:o + 2, :], ident[:]
                        )
                        nc.vector.tensor_copy(
                            ytn[:, p, s * 512 + ti * 128:s * 512 + (ti + 1) * 128],
                            yt[:],
                        )

        # ---- output projection (partial), interleaved per strip ----
        def emit_proj(s, slots=None, use_act=False):
            if slots is None:
                slots = [(qkv_ps, "qkv", 1), (st_ps, "yt", 1)]
            for ti in range(4):
                t = 4 * s + ti
                ot = epool.tile([128, 1024], BF16, name=f"ot_{t}", tag="ot", bufs=7)
                for n in range(2):
                    pool_, tag_, bufs_ = slots[(2 * ti + n) % len(slots)]
                    op = pool_.tile(
                        [128, 512], F32, name=f"op_{t}_{n}", tag=tag_, bufs=bufs_
                    )
                    for f in range(2):
                        nc.tensor.matmul(
                            op[:],
                            ytn[:, f, t * 128:(t + 1) * 128],
                            wp[:, f, n * 512:(n + 1) * 512],
                            start=(f == 0),
                            stop=(f == 1),
                        )
                    if use_act and n == 1:
                        nc.scalar.copy(ot[:, n * 512:(n + 1) * 512], op[:])
                    else:
                        nc.vector.tensor_copy(ot[:, n * 512:(n + 1) * 512], op[:])
                    if not use_act:
                        nc.sync.dma_start(
                            out_d[t * 128:(t + 1) * 128, n * 512:(n + 1) * 512],
                            ot[:, n * 512:(n + 1) * 512],
                        )
                if use_act and ti == 3:
                    nc.sync.dma_start(
                        out_d[t * 128:(t + 1) * 128, 0:512], ot[:, 0:512]
                    )
                    nc.scalar.dma_start(
                        out_d[t * 128:(t + 1) * 128, 512:1024], ot[:, 512:1024]
                    )
                elif use_act:
                    dma = nc.sync.dma_start if ti % 2 == 0 else nc.scalar.dma_start
                    dma(out_d[t * 128:(t + 1) * 128, :], ot[:])

        # ---- skewed software pipeline ----
        # per slot: attention first (highest priority), next strip's QKV and
        # the previous strip's projection as PE stall-filler
        emit_qkv(0, xs0)
        xss = {1: fetch_x(1)} if NST > 1 else {}
        pts = {0: emit_S(0)}
        for s in range(NST):
            if s + 2 < NST:
                xss[s + 2] = fetch_x(s + 2)
            if s + 1 < NST:
                emit_qkv(s + 1, xss.pop(s + 1))
                pts[s + 1] = emit_S(s + 1)
            if s >= 1:
                emit_proj(s - 1)
            emit_PV(s, pts.pop(s))
        # final projection: rotate over the now-free attention PSUM slots so
        # the tail pipeline isn't serialized on a single bank
        emit_proj(
            NST - 1,
            # explicit per-chain slots: early t-tiles may use the y banks
            # (still draining), late t-tiles use only st/qkv banks
            slots=[
                (y_ps, "y", 2), (y_ps, "y", 2),
                (st_ps, "st", 2), (st_ps, "st", 2),
                (qkv_ps, "qkv", 1), (st_ps, "st", 2),
                (st_ps, "st", 2), (qkv_ps, "qkv", 1),
            ],
            use_act=True,
        )

    nc.compile()
    return nc


def _prep_inputs(x, W_attn, b_attn, W_proj):
    """Per-core input maps. Core k: batch k//4, head-group k%4."""
    assert np.allclose(b_attn, 0.0), "nonzero b_attn not supported by this kernel"
    scale = 1.0 / np.sqrt(np.float32(HD))

    mask = (np.arange(128)[:, None] <= np.arange(128)[None, :]).astype(NP_BF16)
    ones = np.ones((128, 1), dtype=NP_BF16)
    ident = np.eye(128, dtype=NP_BF16)

    def lhsT_tiles(w):
        # [C, 128] -> [128, CKT, 128] with [p, t, c] = w[t*128+p, c]
        return np.ascontiguousarray(w.reshape(CKT, 128, 128).transpose(1, 0, 2))

    in_maps = []
    for core in range(NCORES):
        b = core // 4
        g = core % 4
        heads = [4 * g + i for i in range(HEADS_PER_CORE)]
        # [128, CKT, T] with [p, kc, t] = x^T[kc*128+p, t]
        xt = np.ascontiguousarray(
            x[b].T.reshape(CKT, 128, T).transpose(1, 0, 2)
        ).astype(NP_BF16)

        def w_slice(base, hs, sc=1.0):
            cols = np.concatenate(
                [np.arange(base + h * HD, base + (h + 1) * HD) for h in hs]
            )
            return np.ascontiguousarray(W_attn[:, cols]) * sc

        wq = np.stack(
            [lhsT_tiles(w_slice(0, heads[2 * p:2 * p + 2], scale)) for p in range(PAIRS)], axis=1
        ).astype(NP_BF16)  # [128, PAIRS, CKT, 128]
        wk = np.stack(
            [lhsT_tiles(w_slice(C, heads[2 * p:2 * p + 2])) for p in range(PAIRS)], axis=1
        ).astype(NP_BF16)
        # v weights in natural rhs layout: [128, CKT, 256], [p, kc, d] = Wv[kc*128+p, d]
        wv_cols = w_slice(2 * C, heads)  # [C, 256]
        wv = np.ascontiguousarray(
            wv_cols.reshape(CKT, 128, 256).transpose(1, 0, 2)
        ).astype(NP_BF16)
        # W_proj rows for this head group: [256, C] -> [128, 2, C]
        wp_rows = W_proj[heads[0] * HD:(heads[-1] + 1) * HD, :]
        wp = np.ascontiguousarray(
            wp_rows.reshape(2, 128, C).transpose(1, 0, 2)
        ).astype(NP_BF16)

        in_maps.append(
            {
                "xt": np.ascontiguousarray(xt),
                "wq": np.ascontiguousarray(wq),
                "wk": np.ascontiguousarray(wk),
                "wv": np.ascontiguousarray(wv),
                "wp": np.ascontiguousarray(wp),
                "mask": np.ascontiguousarray(mask),
                "ones": ones,
                "ident": ident,
            }
        )
    return in_maps


def kernel(x, W_attn, b_attn, W_proj, b_proj, _want_results=False, _spmd_kwargs=None):
    x = np.asarray(x, dtype=np.float32)
    W_attn = np.asarray(W_attn, dtype=np.float32)
    b_attn = np.asarray(b_attn, dtype=np.float32)
    W_proj = np.asarray(W_proj, dtype=np.float32)
    b_proj = np.asarray(b_proj, dtype=np.float32)

    if "nc" not in _CACHE:
        _CACHE["nc"] = _build()
    nc = _CACHE["nc"]

    in_maps = _prep_inputs(x, W_attn, b_attn, W_proj)
    kw = dict(_spmd_kwargs or {})
    res = run_bass_kernel_spmd(nc, in_maps, list(range(NCORES)), **kw)

    out = np.zeros((B, T, C), dtype=np.float32)
    for core in range(NCORES):
        out[core // 4] += np.asarray(res.results[core]["out"], dtype=np.float32)
    out += b_proj[None, None, :]
    if _want_results:
        return out, res
    return out
